# revision 45
# baseline (speedup 1.0000x reference)
"""ContraFace loss kernel for 8 TRN2 NeuronCores.

Strategy (v3): row-shard the [B, B] cosine matrix across 8 cores (1024 rows
per core). Two SPMD launches:

  Program 1 (norms, ~16us): each core computes, for its own 1024-row shards,
  rn1 = 1/||f1_i||, rn2 = 1/||f2_j|| and the own-row dots ps_i = f1_i.f2_i
  (bf16 inputs, fp32 accums split across ACT squares and DVE stt dots;
  Newton-Raphson rsqrt on DVE/Pool). The host gathers the 8 rn2 shards (pure
  unshard/reshape) so every core can receive the full rn2 row for program 2
  -- this shards the O(B*D) column-norm work 8x without device collectives.

  Program 2 (main, ~80us): per core, compute raw_ij = f1q_i . f2n8_j for its
  1024 rows against all 8192 columns with fp8e4m3 DoubleRow matmuls (256-deep
  contraction per instruction at 0.5 cycles/row -> 4x the fp32r PE rate).
  f2n8 is built on device: f2.T arrives as fp8 panels, Pool/DVE multiply each
  column by bf16(16*rn2_j) and recast to fp8. Per [128 x 2048] PSUM group:
    - ACT: exp with per-partition scale (S/16)*rn1_i, accum_out row-sums
      (the steady-state bottleneck at ~2.08us/group; ACT is the only PSUM
      reader so the 2-deep PSUM pipeline releases quickly)
    - DVE: group max over the monotone bf16 exp values: two pairwise
      tensor_tensor(max) folds 2048->512 plus a tensor_reduce
  Engine placement is constrained by the real ISA: Pool(GpSIMD) only runs
  tensor_tensor(mult)/tensor_scalar(const)/memset, and DVE's
  tensor_tensor_reduce faults at runtime, so folds live on DVE and panel
  scales on Pool. No same-label masking on device: the host removes the
  diagonal and same-label terms exactly by replicating the device's fp8/exp
  arithmetic (ml_dtypes e4m3 matches the DVE/Pool cast bit-exactly) for the
  ~25k affected entries, and recomputes the handful of groups whose max is
  contaminated.

Host combine (float64): pos from exact bf16 dots, neg from corrected group
maxes (cos = ln(max_exp)/S), m EMA, cross-entropy mean.
"""

import sys

sys.path.insert(0, "/opt/trn_rl_repo")

import numpy as np
import ml_dtypes
from contextlib import ExitStack

from concourse import bass, bacc, tile
from concourse.bass_utils import run_bass_kernel_spmd
import concourse.mybir as mybir

dt = mybir.dt
Alu = mybir.AluOpType
Act = mybir.ActivationFunctionType

B, D = 8192, 512
NCORES = 8
BS = B // NCORES          # 1024 rows per core
MT = BS // 128            # 8 m-blocks per core
GW = 2048                 # psum group width
NJG = B // GW             # 4 column groups
PW = 1024                 # f2 panel width
NPAN = B // PW            # 8 panels
S = 64.0
EMA = 0.99
FS = 16.0                 # f2n8 pre-scale (fp8 dynamic range)

_prog_cache = {}


def _nr_rsqrt(eng, pool, dst, ssq_ap, w, tag=""):
    """rsqrt via Newton-Raphson (no ACT table). Seed ~rsqrt(D) is valid for
    L2^2 of D-dim unit-variance gaussian rows; 4 iterations -> fp32 exact."""
    y2 = pool.tile([128, 16], dt.float32, tag="nr_y2" + tag)
    tt = pool.tile([128, 16], dt.float32, tag="nr_t" + tag)
    eng.memset(dst, float(D) ** -0.5)
    for _ in range(4):
        eng.tensor_tensor(out=y2[:, :w], in0=dst, in1=dst, op=Alu.mult)
        eng.tensor_tensor(out=tt[:, :w], in0=ssq_ap, in1=y2[:, :w], op=Alu.mult)
        eng.tensor_scalar(out=tt[:, :w], in0=tt[:, :w], scalar1=-0.5,
                          scalar2=1.5, op0=Alu.mult, op1=Alu.add)
        eng.tensor_tensor(out=dst, in0=dst, in1=tt[:, :w], op=Alu.mult)


def _build_prog1():
    """Per-core shard norms: rn2, rn1, ps. Output st1 [128, 24] fp32:
    [:, 0:8] rn2, [:, 8:16] rn1, [:, 16:24] ps."""
    nc = bacc.Bacc(None)
    f1s_d = nc.declare_dram_parameter("f1s", [BS, D], dt.bfloat16, isOutput=False)
    f2s_d = nc.declare_dram_parameter("f2s", [BS, D], dt.bfloat16, isOutput=False)
    st1_d = nc.declare_dram_parameter("st1", [128, 16], dt.float32, isOutput=True)
    f1s_v = f1s_d[:].rearrange("(m p) d -> p m d", p=128)
    f2s_v = f2s_d[:].rearrange("(m p) d -> p m d", p=128)

    with tile.TileContext(nc) as tc, ExitStack() as ctx:
        cst = ctx.enter_context(tc.tile_pool(name="cst", bufs=1))
        f2t = cst.tile([128, MT, D], dt.bfloat16)
        f1t = cst.tile([128, MT, D], dt.bfloat16)
        # interleave quarters on two queues; f1 first (it feeds the longer
        # DVE chain of squares + pos dots)
        for qt in range(4):
            sl = slice(qt * 2, qt * 2 + 2)
            nc.gpsimd.dma_start(f1t[:, sl, :], f1s_v[:, sl, :])
            nc.sync.dma_start(f2t[:, sl, :], f2s_v[:, sl, :])

        st1 = cst.tile([128, 16], dt.float32)
        rn2_s = st1[:, 0:8]
        rn1_s = st1[:, 8:16]

        ssq2 = cst.tile([128, MT], dt.float32)
        ssq1 = cst.tile([128, MT], dt.float32)
        sqd = cst.tile([128, D], dt.bfloat16, tag="sqd")
        dots = cst.tile([128, D], dt.bfloat16, tag="dots")
        # ACT: f2 squares; DVE: f1 squares (pos dots live in program 2's
        # steady-state DVE slack); NRs on Pool off both tails
        for m in range(MT):
            nc.scalar.activation(sqd[:], f2t[:, m, :], Act.Square,
                                 accum_out=ssq2[:, m : m + 1])
        for m in range(MT):
            nc.vector.scalar_tensor_tensor(
                out=dots[:], in0=f1t[:, m, :], scalar=1.0,
                in1=f1t[:, m, :], op0=Alu.mult, op1=Alu.mult,
                accum_out=ssq1[:, m : m + 1])
        _nr_rsqrt(nc.gpsimd, cst, rn1_s, ssq1[:], MT, tag="a")
        _nr_rsqrt(nc.gpsimd, cst, rn2_s, ssq2[:], MT, tag="b")
        nc.sync.dma_start(st1_d[:], st1[:])

    if not nc.is_finalized():
        nc.finalize()
    return nc


def _build_prog2():
    """Main program: fp8 DoubleRow matmul + exp/max stats for 1024 rows.
    Output st2 [128, 64] fp32: [:, 0:32] group max (g = jg*MT + m),
    [:, 32:64] group sumexp."""
    nc = bacc.Bacc(None)
    f1t8_d = nc.declare_dram_parameter("f1t8", [128, 2, 2, BS], dt.float8e4, isOutput=False)
    rn1_d = nc.declare_dram_parameter("rn1", [128, MT], dt.float32, isOutput=False)
    f2tb_d = nc.declare_dram_parameter("f2tb", [128, 2, 2, B], dt.float8e4, isOutput=False)
    rn2bc_d = nc.declare_dram_parameter("rn2bc", [128, B], dt.bfloat16, isOutput=False)
    f1n_d = nc.declare_dram_parameter("f1n", [BS, D], dt.bfloat16, isOutput=False)
    f2sb_d = nc.declare_dram_parameter("f2sb", [BS, D], dt.bfloat16, isOutput=False)
    st2_d = nc.declare_dram_parameter("st2", [128, 74], dt.float32, isOutput=True)
    f1n_v = f1n_d[:].rearrange("(m p) d -> p m d", p=128)
    f2sb_v = f2sb_d[:].rearrange("(m p) d -> p m d", p=128)

    with tile.TileContext(nc) as tc, ExitStack() as ctx:
        cst = ctx.enter_context(tc.tile_pool(name="cst", bufs=1))
        pan8 = ctx.enter_context(tc.tile_pool(name="pan8", bufs=4))
        panb = ctx.enter_context(tc.tile_pool(name="panb", bufs=2))
        panr = ctx.enter_context(tc.tile_pool(name="panr", bufs=2))
        exq = ctx.enter_context(tc.tile_pool(name="exq", bufs=3))
        fld1 = ctx.enter_context(tc.tile_pool(name="fld1", bufs=3))
        psm = ctx.enter_context(
            tc.tile_pool(name="psm", bufs=2, space=bass.MemorySpace.PSUM)
        )

        st2 = cst.tile([128, 74], dt.float32)
        mx_s = st2[:, 0:32]
        se_s = st2[:, 32:64]
        xb_s = st2[:, 64:66]  # second-half stats of the split first group
        ps_s = st2[:, 66:74]

        f2n8s = {}
        panb_t = {}
        panr_t = {}

        def emit_panel_alloc(h):
            fb = panb.tile([128, 2, 2, PW], dt.float8e4, tag="fb")
            rb = panr.tile([128, PW], dt.bfloat16, tag="rb")
            p8 = pan8.tile([128, 2, 2, PW], dt.float8e4, tag="p8")
            panb_t[h] = fb
            panr_t[h] = rb
            f2n8s[h] = p8

        def emit_panel_dma(h):
            nc.sync.dma_start(panb_t[h][:], f2tb_d[:, :, :, h * PW : (h + 1) * PW])
            nc.sync.dma_start(panr_t[h][:], rn2bc_d[:, h * PW : (h + 1) * PW])

        def emit_panel_scale(h, c, i):
            # prologue panels split DVE/Pool (fill latency); steady panels
            # all on Pool (tt-mult is Pool-legal; DVE carries the ex folds)
            eng = nc.vector if (h < 2 and c == 0) else nc.gpsimd
            eng.tensor_tensor(
                out=f2n8s[h][:, c, i, :],
                in0=panb_t[h][:, c, i, :],
                in1=panr_t[h][:],
                op=Alu.mult,
            )

        # ---- prologue: piecewise DMAs for panels 0-1 on three issue queues
        # (SP / Pool-SWDGE / ACT) so the transfer stream, not the SP
        # sequencer's per-DMA issue cost, bounds the fill.
        emit_panel_alloc(0)
        emit_panel_alloc(1)
        f1t8 = cst.tile([128, 2, 2, BS], dt.float8e4)
        rn1t = cst.tile([128, MT], dt.float32)
        srn1 = cst.tile([128, MT], dt.float32)

        nc.scalar.dma_start(rn1t[:], rn1_d[:])
        nc.scalar.dma_start(f1t8[:], f1t8_d[:])
        q = {0: nc.sync, 1: nc.gpsimd}
        for h in (0, 1):
            q[h].dma_start(panr_t[h][:], rn2bc_d[:, h * PW : (h + 1) * PW])
        for h in (0, 1):
            for c in range(2):
                for i in range(2):
                    q[h].dma_start(
                        panb_t[h][:, c, i, :],
                        f2tb_d[:, c, i, h * PW : (h + 1) * PW],
                    )
        for h in (0, 1):
            for c in range(2):
                for i in range(2):
                    emit_panel_scale(h, c, i)
        nc.gpsimd.tensor_scalar(out=srn1[:], in0=rn1t[:], scalar1=S / FS,
                                scalar2=None, op0=Alu.mult)
        f1n_t = cst.tile([128, MT, D], dt.bfloat16)
        f2sb_t = cst.tile([128, MT, D], dt.bfloat16)
        dots = cst.tile([128, D], dt.bfloat16, tag="dots")

        def emit_pos_dot(m):
            nc.vector.scalar_tensor_tensor(
                out=dots[:], in0=f1n_t[:, m, :], scalar=1.0,
                in1=f2sb_t[:, m, :], op0=Alu.mult, op1=Alu.mult,
                accum_out=ps_s[:, m : m + 1])

        def emit_half_group(half, mxo, seo):
            # half 0 reads panel 0, half 1 reads panel 1 (first group only:
            # lets the first exp start before panel 1 is scaled)
            acc = psm.tile([128, GW], dt.float32, tag="acc")
            ha = acc[:, half * 1024 : half * 1024 + 1024]
            pnl = f2n8s[half]
            for s in range(2):
                for c in range(2):
                    nc.tensor.matmul(
                        ha[:, s * 512 : (s + 1) * 512],
                        f1t8[:, c, :, 0:128],
                        pnl[:, c, :, s * 512 : (s + 1) * 512],
                        start=(c == 0),
                        stop=(c == 1),
                        perf_mode=mybir.MatmulPerfMode.DoubleRow,
                    )
            ex = exq.tile([128, GW], dt.bfloat16, tag="ex")
            nc.scalar.activation(ex[:, :1024], ha, Act.Exp, bias=0.0,
                                 scale=srn1[:, 0:1], accum_out=seo)
            fa = fld1.tile([128, GW // 2], dt.bfloat16, tag="fa")
            nc.vector.tensor_tensor(out=fa[:, :512], in0=ex[:, :512],
                                    in1=ex[:, 512:1024], op=Alu.max)
            fb = fld1.tile([128, GW // 4], dt.bfloat16, tag="fb")
            nc.vector.tensor_tensor(out=fb[:, :256], in0=fa[:, :256],
                                    in1=fa[:, 256:512], op=Alu.max)
            nc.vector.tensor_reduce(out=mxo, in_=fb[:, :256],
                                    axis=mybir.AxisListType.X, op=Alu.max)

        def emit_group(jg, m):
            acc = psm.tile([128, GW], dt.float32, tag="acc")
            for s in range(4):
                pnl = f2n8s[2 * jg + s // 2]
                off = (s % 2) * 512
                for c in range(2):
                    nc.tensor.matmul(
                        acc[:, s * 512 : (s + 1) * 512],
                        f1t8[:, c, :, m * 128 : (m + 1) * 128],
                        pnl[:, c, :, off : off + 512],
                        start=(c == 0),
                        stop=(c == 1),
                        perf_mode=mybir.MatmulPerfMode.DoubleRow,
                    )
            g = jg * MT + m
            ex = exq.tile([128, GW], dt.bfloat16, tag="ex")
            nc.scalar.activation(ex[:], acc[:], Act.Exp, bias=0.0,
                                 scale=srn1[:, m : m + 1],
                                 accum_out=se_s[:, g : g + 1])
            # group max taken over the monotone exp values (bf16) on DVE;
            # only ACT touches PSUM, keeping the release chain short
            fa = fld1.tile([128, GW // 2], dt.bfloat16, tag="fa")
            nc.vector.tensor_tensor(out=fa[:], in0=ex[:, : GW // 2],
                                    in1=ex[:, GW // 2 :], op=Alu.max)
            fb = fld1.tile([128, GW // 4], dt.bfloat16, tag="fb")
            nc.vector.tensor_tensor(out=fb[:], in0=fa[:, : GW // 4],
                                    in1=fa[:, GW // 4 :], op=Alu.max)
            nc.vector.tensor_reduce(out=mx_s[:, g : g + 1], in_=fb[:],
                                    axis=mybir.AxisListType.X, op=Alu.max)

        for jg in range(NJG):
            for m in range(MT):
                if jg == 0 and m == 0:
                    emit_half_group(0, mx_s[:, 0:1], se_s[:, 32 - 32 : 33 - 32])
                    emit_half_group(1, xb_s[:, 0:1], xb_s[:, 1:2])
                else:
                    emit_group(jg, m)
                if jg + 1 < NJG:
                    # stage panels 2*(jg+1), 2*(jg+1)+1 under this group sweep
                    if m == 0:
                        emit_panel_alloc(2 * (jg + 1))
                        emit_panel_dma(2 * (jg + 1))
                    elif m == 1:
                        emit_panel_alloc(2 * (jg + 1) + 1)
                        emit_panel_dma(2 * (jg + 1) + 1)
                    elif 2 <= m < 6:
                        h = 2 * (jg + 1) + (m - 2) // 2
                        ci = (m - 2) % 2
                        emit_panel_scale(h, ci, 0)
                        emit_panel_scale(h, ci, 1)
                if jg == 0 and m == 2:
                    # fetch pos-dot operands after the prologue stream drains
                    nc.scalar.dma_start(f1n_t[:], f1n_v)
                elif jg == 0 and m == 3:
                    nc.scalar.dma_start(f2sb_t[:], f2sb_v)
                if jg in (1, 2) and m % 2 == 0:
                    # pos dots in DVE steady-state slack, spread to
                    # alternating slots so the fold pipeline absorbs them
                    emit_pos_dot((jg - 1) * 4 + m // 2)
                if jg == NJG - 1 and m == 0:
                    # drain split: ship groups 0-23 early; only the last jg's
                    # columns ride the final (small) DMA
                    nc.sync.dma_start(st2_d[:, 0:24], st2[:, 0:24])
                    nc.sync.dma_start(st2_d[:, 32:56], st2[:, 32:56])

        nc.sync.dma_start(st2_d[:, 24:32], st2[:, 24:32])
        nc.sync.dma_start(st2_d[:, 56:74], st2[:, 56:74])

    if not nc.is_finalized():
        nc.finalize()
    return nc


def _get_prog1():
    if "p1" not in _prog_cache:
        _prog_cache["p1"] = _build_prog1()
    return _prog_cache["p1"]


def _get_prog2():
    if "p2" not in _prog_cache:
        _prog_cache["p2"] = _build_prog2()
    return _prog_cache["p2"]


def _prog1_inputs(f1b, f2b):
    return [
        dict(
            f1s=np.ascontiguousarray(f1b[c * BS : (c + 1) * BS]),
            f2s=np.ascontiguousarray(f2b[c * BS : (c + 1) * BS]),
        )
        for c in range(NCORES)
    ]


def _prog2_inputs(f1, f2, rn1, rn2):
    f1q8 = f1.astype(ml_dtypes.float8_e4m3fn)
    f2tb = np.ascontiguousarray(
        f2.T.reshape(2, 2, 128, B).transpose(2, 0, 1, 3).astype(ml_dtypes.float8_e4m3fn)
    )
    rn2bc = np.ascontiguousarray(
        np.broadcast_to(
            (np.float32(FS) * rn2)[None, :].astype(ml_dtypes.bfloat16), (128, B)
        )
    )
    in_maps = []
    for c in range(NCORES):
        sl = slice(c * BS, (c + 1) * BS)
        f1t8 = np.ascontiguousarray(
            f1q8[sl].T.reshape(2, 2, 128, BS).transpose(2, 0, 1, 3)
        )
        # rn1 packed [128, MT]: [p, m] = rn1[c*BS + m*128 + p]
        rn1p = np.ascontiguousarray(rn1[sl].reshape(MT, 128).T.astype(np.float32))
        in_maps.append(dict(
            f1t8=f1t8, rn1=rn1p, f2tb=f2tb, rn2bc=rn2bc,
            f1n=np.ascontiguousarray(f1.astype(ml_dtypes.bfloat16)[sl]),
            f2sb=np.ascontiguousarray(f2.astype(ml_dtypes.bfloat16)[sl]),
        ))
    return in_maps


def _host_combine(f1, f2, lab, rn1, rn2, ps, mx, se):
    """Exact host-side unmasking + final CE in float64.

    Replicates the device fp8 chain bit-exactly (ml_dtypes e4m3 == DVE cast)
    for the diagonal and same-label entries so their exp/max contributions can
    be removed from the raw (unmasked) device stats.
    """
    ar = np.arange(B)
    srn1 = (np.float32(S / FS) * rn1.astype(np.float32)).astype(np.float32)

    # device-equivalent operands
    f1q32 = f1.astype(ml_dtypes.float8_e4m3fn).astype(np.float32)
    f2tb32 = f2.T.astype(ml_dtypes.float8_e4m3fn).astype(np.float32)
    rn2bc = (
        (np.float32(FS) * rn2.astype(np.float32))
        .astype(ml_dtypes.bfloat16)
        .astype(np.float32)
    )
    f2n8 = (f2tb32 * rn2bc[None, :]).astype(ml_dtypes.float8_e4m3fn).astype(np.float32)

    # same-label off-diagonal ordered pairs
    order = np.argsort(lab, kind="stable")
    lab_s = lab[order]
    bounds = np.flatnonzero(np.r_[True, lab_s[1:] != lab_s[:-1], True])
    pi, pj = [], []
    for a, b in zip(bounds[:-1], bounds[1:]):
        if b - a > 1:
            mem = order[a:b]
            ii, jj = np.meshgrid(mem, mem, indexing="ij")
            msk = ii != jj
            pi.append(ii[msk])
            pj.append(jj[msk])
    if pi:
        pi = np.concatenate(pi)
        pj = np.concatenate(pj)
    else:
        pi = np.zeros(0, np.int64)
        pj = np.zeros(0, np.int64)

    raw_sl = np.einsum("kd,dk->k", f1q32[pi], f2n8[:, pj], dtype=np.float32)
    raw_diag = np.einsum("id,di->i", f1q32, f2n8, dtype=np.float32)

    # sumexp correction: masked entries (same-label offdiag) count as exp(0)=1;
    # the diagonal is re-added on the host from the exact pos. (The device
    # accumulates fp32 exp values, so plain np.exp replicates it.)
    t_sl = np.exp(srn1[pi].astype(np.float64) * raw_sl.astype(np.float64))
    t_diag = np.exp(srn1.astype(np.float64) * raw_diag.astype(np.float64))
    cnt = np.bincount(pi, minlength=B).astype(np.float64)
    sumoff = (
        se.sum(axis=1)
        - t_diag
        - np.bincount(pi, weights=t_sl, minlength=B)
        + cnt
    )

    # group-max correction. Device group maxes are max_j bf16(exp(S*cos_ij))
    # (monotone in cos). Exclude contaminated entries: where a contaminated
    # ex-value could be the group max, recompute that group's masked max
    # exactly; otherwise cos_max = ln(mx)/S.
    def dev_ex(i_arr, raw_arr):
        v = np.exp(
            (srn1[i_arr].astype(np.float32) * raw_arr).astype(np.float32)
        ).astype(ml_dtypes.bfloat16)
        return v.astype(np.float64)

    ex_sl = dev_ex(pi, raw_sl)
    ex_diag = dev_ex(ar, raw_diag)
    cmax = np.full((B, NJG), -np.inf)
    np.maximum.at(cmax, (pi, pj // GW), ex_sl)
    np.maximum.at(cmax, (ar, ar // GW), ex_diag)
    suspect = mx <= cmax * 1.02  # group max may be a contaminated entry
    with np.errstate(divide="ignore"):
        cosg = np.where(mx > 0, np.log(np.maximum(mx, 1e-300)) / S, -np.inf)
    cosg = np.where(suspect, -np.inf, cosg)
    si, sg = np.nonzero(suspect)
    for i, g in zip(si, sg):
        sl_ = slice(g * GW, (g + 1) * GW)
        row = f1q32[i] @ f2n8[:, sl_]
        mask = np.ones(GW, bool)
        mask[np.flatnonzero((lab[sl_] == lab[i]) | (ar[sl_] == i))] = False
        cosg[i, g] = np.float64(srn1[i]) * row[mask].max() / S
    neg = np.maximum(0.0, cosg.max(axis=1))

    pos = np.clip(ps * rn1 * rn2.astype(np.float64), -1.0, 1.0)
    m = EMA * np.mean(pos - neg)
    z = S * (pos - m)
    loss = np.mean(np.log(sumoff + np.exp(z)) - z)
    return np.float32(loss)


def _unpack_st1(res):
    rn2 = np.empty(B, np.float32)
    rn1 = np.empty(B, np.float64)
    for c in range(NCORES):
        st = np.asarray(res[c]["st1"], np.float64)
        sl = slice(c * BS, (c + 1) * BS)
        rn2[sl] = st[:, 0:8].T.reshape(BS).astype(np.float32)
        rn1[sl] = st[:, 8:16].T.reshape(BS)
    return rn1, rn2


def _unpack_st2(res):
    mx = np.empty((B, NJG), np.float64)
    se = np.empty((B, NJG), np.float64)
    ps = np.empty(B, np.float64)
    for c in range(NCORES):
        st = np.asarray(res[c]["st2"], np.float64)
        # merge the split first group (jg0, m0): cols 64/65 hold its
        # panel-1 half max / sumexp
        st = st.copy()
        st[:, 0] = np.maximum(st[:, 0], st[:, 64])
        st[:, 32] = st[:, 32] + st[:, 65]
        sl = slice(c * BS, (c + 1) * BS)
        mx[sl] = st[:, 0:32].reshape(128, NJG, MT).transpose(2, 0, 1).reshape(BS, NJG)
        se[sl] = st[:, 32:64].reshape(128, NJG, MT).transpose(2, 0, 1).reshape(BS, NJG)
        ps[sl] = st[:, 66:74].T.reshape(BS)
    return mx, se, ps


def kernel(feature1, feature2, label):
    f1 = np.ascontiguousarray(np.asarray(feature1, dtype=np.float32))
    f2 = np.ascontiguousarray(np.asarray(feature2, dtype=np.float32))
    lab = np.asarray(label)

    f1b = f1.astype(ml_dtypes.bfloat16)
    f2b = f2.astype(ml_dtypes.bfloat16)

    # program 1: per-core shard norms + own-row dots
    out1 = run_bass_kernel_spmd(
        _get_prog1(), _prog1_inputs(f1b, f2b), list(range(NCORES))
    ).results
    rn1, rn2 = _unpack_st1(out1)

    # program 2: main cos/exp/max stats
    out2 = run_bass_kernel_spmd(
        _get_prog2(), _prog2_inputs(f1, f2, rn1, rn2), list(range(NCORES))
    ).results
    mx, se, ps = _unpack_st2(out2)

    return _host_combine(f1, f2, lab, rn1, rn2, ps, mx, se)


# revision 46
# speedup vs baseline: 1.0260x; 1.0260x over previous
"""ContraFace loss kernel for 8 TRN2 NeuronCores.

Strategy (v3): row-shard the [B, B] cosine matrix across 8 cores (1024 rows
per core). Two SPMD launches:

  Program 1 (norms, ~16us): each core computes, for its own 1024-row shards,
  rn1 = 1/||f1_i||, rn2 = 1/||f2_j|| and the own-row dots ps_i = f1_i.f2_i
  (bf16 inputs, fp32 accums split across ACT squares and DVE stt dots;
  Newton-Raphson rsqrt on DVE/Pool). The host gathers the 8 rn2 shards (pure
  unshard/reshape) so every core can receive the full rn2 row for program 2
  -- this shards the O(B*D) column-norm work 8x without device collectives.

  Program 2 (main, ~80us): per core, compute raw_ij = f1q_i . f2n8_j for its
  1024 rows against all 8192 columns with fp8e4m3 DoubleRow matmuls (256-deep
  contraction per instruction at 0.5 cycles/row -> 4x the fp32r PE rate).
  f2n8 is built on device: f2.T arrives as fp8 panels, Pool/DVE multiply each
  column by bf16(16*rn2_j) and recast to fp8. Per [128 x 2048] PSUM group:
    - ACT: exp with per-partition scale (S/16)*rn1_i, accum_out row-sums
      (the steady-state bottleneck at ~2.08us/group; ACT is the only PSUM
      reader so the 2-deep PSUM pipeline releases quickly)
    - DVE: group max over the monotone bf16 exp values: two pairwise
      tensor_tensor(max) folds 2048->512 plus a tensor_reduce
  Engine placement is constrained by the real ISA: Pool(GpSIMD) only runs
  tensor_tensor(mult)/tensor_scalar(const)/memset, and DVE's
  tensor_tensor_reduce faults at runtime, so folds live on DVE and panel
  scales on Pool. No same-label masking on device: the host removes the
  diagonal and same-label terms exactly by replicating the device's fp8/exp
  arithmetic (ml_dtypes e4m3 matches the DVE/Pool cast bit-exactly) for the
  ~25k affected entries, and recomputes the handful of groups whose max is
  contaminated.

Host combine (float64): pos from exact bf16 dots, neg from corrected group
maxes (cos = ln(max_exp)/S), m EMA, cross-entropy mean.
"""

import sys

sys.path.insert(0, "/opt/trn_rl_repo")

import numpy as np
import ml_dtypes
from contextlib import ExitStack

from concourse import bass, bacc, tile
from concourse.bass_utils import run_bass_kernel_spmd
import concourse.mybir as mybir

dt = mybir.dt
Alu = mybir.AluOpType
Act = mybir.ActivationFunctionType

B, D = 8192, 512
NCORES = 8
BS = B // NCORES          # 1024 rows per core
MT = BS // 128            # 8 m-blocks per core
GW = 2048                 # psum group width
NJG = B // GW             # 4 column groups
PW = 1024                 # f2 panel width
NPAN = B // PW            # 8 panels
S = 64.0
EMA = 0.99
FS = 16.0                 # f2n8 pre-scale (fp8 dynamic range)

_prog_cache = {}


def _nr_rsqrt(eng, pool, dst, ssq_ap, w, tag=""):
    """rsqrt via Newton-Raphson (no ACT table). Seed ~rsqrt(D) is valid for
    L2^2 of D-dim unit-variance gaussian rows; 4 iterations -> fp32 exact."""
    y2 = pool.tile([128, 16], dt.float32, tag="nr_y2" + tag)
    tt = pool.tile([128, 16], dt.float32, tag="nr_t" + tag)
    eng.memset(dst, float(D) ** -0.5)
    for _ in range(4):
        eng.tensor_tensor(out=y2[:, :w], in0=dst, in1=dst, op=Alu.mult)
        eng.tensor_tensor(out=tt[:, :w], in0=ssq_ap, in1=y2[:, :w], op=Alu.mult)
        eng.tensor_scalar(out=tt[:, :w], in0=tt[:, :w], scalar1=-0.5,
                          scalar2=1.5, op0=Alu.mult, op1=Alu.add)
        eng.tensor_tensor(out=dst, in0=dst, in1=tt[:, :w], op=Alu.mult)


def _build_prog1():
    """Per-core shard norms: rn2, rn1, ps. Output st1 [128, 24] fp32:
    [:, 0:8] rn2, [:, 8:16] rn1, [:, 16:24] ps."""
    nc = bacc.Bacc(None)
    f1s_d = nc.declare_dram_parameter("f1s", [BS, D], dt.bfloat16, isOutput=False)
    f2s_d = nc.declare_dram_parameter("f2s", [BS, D], dt.bfloat16, isOutput=False)
    st1_d = nc.declare_dram_parameter("st1", [128, 16], dt.float32, isOutput=True)
    f1s_v = f1s_d[:].rearrange("(m p) d -> p m d", p=128)
    f2s_v = f2s_d[:].rearrange("(m p) d -> p m d", p=128)

    with tile.TileContext(nc) as tc, ExitStack() as ctx:
        cst = ctx.enter_context(tc.tile_pool(name="cst", bufs=1))
        f2t = cst.tile([128, MT, D], dt.bfloat16)
        f1t = cst.tile([128, MT, D], dt.bfloat16)
        # interleave quarters on two queues; f1 first (it feeds the longer
        # DVE chain of squares + pos dots)
        for qt in range(4):
            sl = slice(qt * 2, qt * 2 + 2)
            nc.gpsimd.dma_start(f1t[:, sl, :], f1s_v[:, sl, :])
            nc.sync.dma_start(f2t[:, sl, :], f2s_v[:, sl, :])

        st1 = cst.tile([128, 16], dt.float32)
        rn2_s = st1[:, 0:8]
        rn1_s = st1[:, 8:16]

        ssq2 = cst.tile([128, MT], dt.float32)
        ssq1 = cst.tile([128, MT], dt.float32)
        sqd = cst.tile([128, D], dt.bfloat16, tag="sqd")
        dots = cst.tile([128, D], dt.bfloat16, tag="dots")
        # ACT: f2 squares; DVE: f1 squares (pos dots live in program 2's
        # steady-state DVE slack); NRs on Pool off both tails
        for m in range(MT):
            nc.scalar.activation(sqd[:], f2t[:, m, :], Act.Square,
                                 accum_out=ssq2[:, m : m + 1])
        for m in range(MT):
            nc.vector.scalar_tensor_tensor(
                out=dots[:], in0=f1t[:, m, :], scalar=1.0,
                in1=f1t[:, m, :], op0=Alu.mult, op1=Alu.mult,
                accum_out=ssq1[:, m : m + 1])
        _nr_rsqrt(nc.gpsimd, cst, rn1_s, ssq1[:], MT, tag="a")
        _nr_rsqrt(nc.gpsimd, cst, rn2_s, ssq2[:], MT, tag="b")
        nc.sync.dma_start(st1_d[:], st1[:])

    if not nc.is_finalized():
        nc.finalize()
    return nc


def _build_prog2():
    """Main program: fp8 DoubleRow matmul + exp/max stats for 1024 rows.
    Output st2 [128, 64] fp32: [:, 0:32] group max (g = jg*MT + m),
    [:, 32:64] group sumexp."""
    nc = bacc.Bacc(None)
    f1t8_d = nc.declare_dram_parameter("f1t8", [128, 2, 2, BS], dt.float8e4, isOutput=False)
    rn1_d = nc.declare_dram_parameter("rn1", [128, MT], dt.float32, isOutput=False)
    f2tb_d = nc.declare_dram_parameter("f2tb", [128, 2, 2, B], dt.float8e4, isOutput=False)
    rn2bc_d = nc.declare_dram_parameter("rn2bc", [128, B], dt.bfloat16, isOutput=False)
    f1n_d = nc.declare_dram_parameter("f1n", [BS, D], dt.bfloat16, isOutput=False)
    f2sb_d = nc.declare_dram_parameter("f2sb", [BS, D], dt.bfloat16, isOutput=False)
    st2_d = nc.declare_dram_parameter("st2", [128, 74], dt.float32, isOutput=True)
    f1n_v = f1n_d[:].rearrange("(m p) d -> p m d", p=128)
    f2sb_v = f2sb_d[:].rearrange("(m p) d -> p m d", p=128)

    with tile.TileContext(nc) as tc, ExitStack() as ctx:
        cst = ctx.enter_context(tc.tile_pool(name="cst", bufs=1))
        pan8 = ctx.enter_context(tc.tile_pool(name="pan8", bufs=4))
        panb = ctx.enter_context(tc.tile_pool(name="panb", bufs=2))
        panr = ctx.enter_context(tc.tile_pool(name="panr", bufs=2))
        exq = ctx.enter_context(tc.tile_pool(name="exq", bufs=3))
        fld1 = ctx.enter_context(tc.tile_pool(name="fld1", bufs=3))
        psm = ctx.enter_context(
            tc.tile_pool(name="psm", bufs=2, space=bass.MemorySpace.PSUM)
        )

        st2 = cst.tile([128, 74], dt.float32)
        mx_s = st2[:, 0:32]
        se_s = st2[:, 32:64]
        xb_s = st2[:, 64:66]  # second-half stats of the split first group
        ps_s = st2[:, 66:74]

        f2n8s = {}
        panb_t = {}
        panr_t = {}

        def emit_panel_alloc(h):
            fb = panb.tile([128, 2, 2, PW], dt.float8e4, tag="fb")
            rb = panr.tile([128, PW], dt.bfloat16, tag="rb")
            p8 = pan8.tile([128, 2, 2, PW], dt.float8e4, tag="p8")
            panb_t[h] = fb
            panr_t[h] = rb
            f2n8s[h] = p8

        def emit_panel_dma(h):
            nc.sync.dma_start(panb_t[h][:], f2tb_d[:, :, :, h * PW : (h + 1) * PW])
            nc.sync.dma_start(panr_t[h][:], rn2bc_d[:, h * PW : (h + 1) * PW])

        def emit_panel_scale(h, c, i):
            # prologue panels split DVE/Pool (fill latency); steady panels
            # all on Pool (tt-mult is Pool-legal; DVE carries the ex folds)
            eng = nc.vector if (h < 2 and c == 0) else nc.gpsimd
            eng.tensor_tensor(
                out=f2n8s[h][:, c, i, :],
                in0=panb_t[h][:, c, i, :],
                in1=panr_t[h][:],
                op=Alu.mult,
            )

        # ---- prologue: piecewise DMAs for panels 0-1 on three issue queues
        # (SP / Pool-SWDGE / ACT) so the transfer stream, not the SP
        # sequencer's per-DMA issue cost, bounds the fill.
        emit_panel_alloc(0)
        emit_panel_alloc(1)
        f1t8 = cst.tile([128, 2, 2, BS], dt.float8e4)
        rn1t = cst.tile([128, MT], dt.float32)
        srn1 = cst.tile([128, MT], dt.float32)

        nc.scalar.dma_start(rn1t[:], rn1_d[:])
        nc.scalar.dma_start(f1t8[:], f1t8_d[:])
        q = {0: nc.sync, 1: nc.gpsimd}
        for h in (0, 1):
            q[h].dma_start(panr_t[h][:], rn2bc_d[:, h * PW : (h + 1) * PW])
        for h in (0, 1):
            for c in range(2):
                for i in range(2):
                    q[h].dma_start(
                        panb_t[h][:, c, i, :],
                        f2tb_d[:, c, i, h * PW : (h + 1) * PW],
                    )
        for h in (0, 1):
            for c in range(2):
                for i in range(2):
                    emit_panel_scale(h, c, i)
        nc.gpsimd.tensor_scalar(out=srn1[:], in0=rn1t[:], scalar1=S / FS,
                                scalar2=None, op0=Alu.mult)
        f1n_t = cst.tile([128, MT, D], dt.bfloat16)
        f2sb_t = cst.tile([128, MT, D], dt.bfloat16)
        dots = cst.tile([128, D], dt.bfloat16, tag="dots")

        def emit_pos_dot(m):
            nc.vector.scalar_tensor_tensor(
                out=dots[:], in0=f1n_t[:, m, :], scalar=1.0,
                in1=f2sb_t[:, m, :], op0=Alu.mult, op1=Alu.mult,
                accum_out=ps_s[:, m : m + 1])

        def emit_half_group(half, mxo, seo):
            # half 0 reads panel 0, half 1 reads panel 1 (first group only:
            # lets the first exp start before panel 1 is scaled)
            acc = psm.tile([128, GW], dt.float32, tag="acc")
            ha = acc[:, half * 1024 : half * 1024 + 1024]
            pnl = f2n8s[half]
            for s in range(2):
                for c in range(2):
                    nc.tensor.matmul(
                        ha[:, s * 512 : (s + 1) * 512],
                        f1t8[:, c, :, 0:128],
                        pnl[:, c, :, s * 512 : (s + 1) * 512],
                        start=(c == 0),
                        stop=(c == 1),
                        perf_mode=mybir.MatmulPerfMode.DoubleRow,
                    )
            ex = exq.tile([128, GW], dt.bfloat16, tag="ex")
            nc.scalar.activation(ex[:, :1024], ha, Act.Exp, bias=0.0,
                                 scale=srn1[:, 0:1], accum_out=seo)
            fa = fld1.tile([128, GW // 2], dt.bfloat16, tag="fa")
            nc.vector.tensor_tensor(out=fa[:, :512], in0=ex[:, :512],
                                    in1=ex[:, 512:1024], op=Alu.max)
            fb = fld1.tile([128, GW // 4], dt.bfloat16, tag="fb")
            nc.vector.tensor_tensor(out=fb[:, :256], in0=fa[:, :256],
                                    in1=fa[:, 256:512], op=Alu.max)
            nc.vector.tensor_reduce(out=mxo, in_=fb[:, :256],
                                    axis=mybir.AxisListType.X, op=Alu.max)

        def emit_group(jg, m):
            acc = psm.tile([128, GW], dt.float32, tag="acc")
            for s in range(4):
                pnl = f2n8s[2 * jg + s // 2]
                off = (s % 2) * 512
                for c in range(2):
                    nc.tensor.matmul(
                        acc[:, s * 512 : (s + 1) * 512],
                        f1t8[:, c, :, m * 128 : (m + 1) * 128],
                        pnl[:, c, :, off : off + 512],
                        start=(c == 0),
                        stop=(c == 1),
                        perf_mode=mybir.MatmulPerfMode.DoubleRow,
                    )
            g = jg * MT + m
            ex = exq.tile([128, GW], dt.bfloat16, tag="ex")
            nc.scalar.activation(ex[:], acc[:], Act.Exp, bias=0.0,
                                 scale=srn1[:, m : m + 1],
                                 accum_out=se_s[:, g : g + 1])
            # group max taken over the monotone exp values (bf16) on DVE;
            # only ACT touches PSUM, keeping the release chain short
            fa = fld1.tile([128, GW // 2], dt.bfloat16, tag="fa")
            nc.vector.tensor_tensor(out=fa[:], in0=ex[:, : GW // 2],
                                    in1=ex[:, GW // 2 :], op=Alu.max)
            fb = fld1.tile([128, GW // 4], dt.bfloat16, tag="fb")
            nc.vector.tensor_tensor(out=fb[:], in0=fa[:, : GW // 4],
                                    in1=fa[:, GW // 4 :], op=Alu.max)
            nc.vector.tensor_reduce(out=mx_s[:, g : g + 1], in_=fb[:],
                                    axis=mybir.AxisListType.X, op=Alu.max)

        for jg in range(NJG):
            for m in range(MT):
                if jg == 0 and m == 0:
                    emit_half_group(0, mx_s[:, 0:1], se_s[:, 32 - 32 : 33 - 32])
                    emit_half_group(1, xb_s[:, 0:1], xb_s[:, 1:2])
                else:
                    emit_group(jg, m)
                if jg + 1 < NJG:
                    # stage panels 2*(jg+1), 2*(jg+1)+1 under this group sweep
                    if m == 0:
                        emit_panel_alloc(2 * (jg + 1))
                        emit_panel_dma(2 * (jg + 1))
                    elif m == 1:
                        emit_panel_alloc(2 * (jg + 1) + 1)
                        emit_panel_dma(2 * (jg + 1) + 1)
                    elif 2 <= m < 6:
                        h = 2 * (jg + 1) + (m - 2) // 2
                        ci = (m - 2) % 2
                        emit_panel_scale(h, ci, 0)
                        emit_panel_scale(h, ci, 1)
                if jg == 0 and m == 2:
                    # fetch pos-dot operands on the SP queue: DGE work on the
                    # ACT sequencer would block parked exp dispatch mid-fill
                    nc.sync.dma_start(f1n_t[:], f1n_v)
                elif jg == 0 and m == 3:
                    nc.sync.dma_start(f2sb_t[:], f2sb_v)
                if jg in (1, 2) and m % 2 == 0:
                    # pos dots in DVE steady-state slack, spread to
                    # alternating slots so the fold pipeline absorbs them
                    emit_pos_dot((jg - 1) * 4 + m // 2)
                if jg == NJG - 1 and m == 0:
                    # drain split: ship groups 0-23 early; only the last jg's
                    # columns ride the final (small) DMA
                    nc.sync.dma_start(st2_d[:, 0:24], st2[:, 0:24])
                    nc.sync.dma_start(st2_d[:, 32:56], st2[:, 32:56])

        nc.sync.dma_start(st2_d[:, 24:32], st2[:, 24:32])
        nc.sync.dma_start(st2_d[:, 56:74], st2[:, 56:74])

    if not nc.is_finalized():
        nc.finalize()
    return nc


def _get_prog1():
    if "p1" not in _prog_cache:
        _prog_cache["p1"] = _build_prog1()
    return _prog_cache["p1"]


def _get_prog2():
    if "p2" not in _prog_cache:
        _prog_cache["p2"] = _build_prog2()
    return _prog_cache["p2"]


def _prog1_inputs(f1b, f2b):
    return [
        dict(
            f1s=np.ascontiguousarray(f1b[c * BS : (c + 1) * BS]),
            f2s=np.ascontiguousarray(f2b[c * BS : (c + 1) * BS]),
        )
        for c in range(NCORES)
    ]


def _prog2_inputs(f1, f2, rn1, rn2):
    f1q8 = f1.astype(ml_dtypes.float8_e4m3fn)
    f2tb = np.ascontiguousarray(
        f2.T.reshape(2, 2, 128, B).transpose(2, 0, 1, 3).astype(ml_dtypes.float8_e4m3fn)
    )
    rn2bc = np.ascontiguousarray(
        np.broadcast_to(
            (np.float32(FS) * rn2)[None, :].astype(ml_dtypes.bfloat16), (128, B)
        )
    )
    in_maps = []
    for c in range(NCORES):
        sl = slice(c * BS, (c + 1) * BS)
        f1t8 = np.ascontiguousarray(
            f1q8[sl].T.reshape(2, 2, 128, BS).transpose(2, 0, 1, 3)
        )
        # rn1 packed [128, MT]: [p, m] = rn1[c*BS + m*128 + p]
        rn1p = np.ascontiguousarray(rn1[sl].reshape(MT, 128).T.astype(np.float32))
        in_maps.append(dict(
            f1t8=f1t8, rn1=rn1p, f2tb=f2tb, rn2bc=rn2bc,
            f1n=np.ascontiguousarray(f1.astype(ml_dtypes.bfloat16)[sl]),
            f2sb=np.ascontiguousarray(f2.astype(ml_dtypes.bfloat16)[sl]),
        ))
    return in_maps


def _host_combine(f1, f2, lab, rn1, rn2, ps, mx, se):
    """Exact host-side unmasking + final CE in float64.

    Replicates the device fp8 chain bit-exactly (ml_dtypes e4m3 == DVE cast)
    for the diagonal and same-label entries so their exp/max contributions can
    be removed from the raw (unmasked) device stats.
    """
    ar = np.arange(B)
    srn1 = (np.float32(S / FS) * rn1.astype(np.float32)).astype(np.float32)

    # device-equivalent operands
    f1q32 = f1.astype(ml_dtypes.float8_e4m3fn).astype(np.float32)
    f2tb32 = f2.T.astype(ml_dtypes.float8_e4m3fn).astype(np.float32)
    rn2bc = (
        (np.float32(FS) * rn2.astype(np.float32))
        .astype(ml_dtypes.bfloat16)
        .astype(np.float32)
    )
    f2n8 = (f2tb32 * rn2bc[None, :]).astype(ml_dtypes.float8_e4m3fn).astype(np.float32)

    # same-label off-diagonal ordered pairs
    order = np.argsort(lab, kind="stable")
    lab_s = lab[order]
    bounds = np.flatnonzero(np.r_[True, lab_s[1:] != lab_s[:-1], True])
    pi, pj = [], []
    for a, b in zip(bounds[:-1], bounds[1:]):
        if b - a > 1:
            mem = order[a:b]
            ii, jj = np.meshgrid(mem, mem, indexing="ij")
            msk = ii != jj
            pi.append(ii[msk])
            pj.append(jj[msk])
    if pi:
        pi = np.concatenate(pi)
        pj = np.concatenate(pj)
    else:
        pi = np.zeros(0, np.int64)
        pj = np.zeros(0, np.int64)

    raw_sl = np.einsum("kd,dk->k", f1q32[pi], f2n8[:, pj], dtype=np.float32)
    raw_diag = np.einsum("id,di->i", f1q32, f2n8, dtype=np.float32)

    # sumexp correction: masked entries (same-label offdiag) count as exp(0)=1;
    # the diagonal is re-added on the host from the exact pos. (The device
    # accumulates fp32 exp values, so plain np.exp replicates it.)
    t_sl = np.exp(srn1[pi].astype(np.float64) * raw_sl.astype(np.float64))
    t_diag = np.exp(srn1.astype(np.float64) * raw_diag.astype(np.float64))
    cnt = np.bincount(pi, minlength=B).astype(np.float64)
    sumoff = (
        se.sum(axis=1)
        - t_diag
        - np.bincount(pi, weights=t_sl, minlength=B)
        + cnt
    )

    # group-max correction. Device group maxes are max_j bf16(exp(S*cos_ij))
    # (monotone in cos). Exclude contaminated entries: where a contaminated
    # ex-value could be the group max, recompute that group's masked max
    # exactly; otherwise cos_max = ln(mx)/S.
    def dev_ex(i_arr, raw_arr):
        v = np.exp(
            (srn1[i_arr].astype(np.float32) * raw_arr).astype(np.float32)
        ).astype(ml_dtypes.bfloat16)
        return v.astype(np.float64)

    ex_sl = dev_ex(pi, raw_sl)
    ex_diag = dev_ex(ar, raw_diag)
    cmax = np.full((B, NJG), -np.inf)
    np.maximum.at(cmax, (pi, pj // GW), ex_sl)
    np.maximum.at(cmax, (ar, ar // GW), ex_diag)
    suspect = mx <= cmax * 1.02  # group max may be a contaminated entry
    with np.errstate(divide="ignore"):
        cosg = np.where(mx > 0, np.log(np.maximum(mx, 1e-300)) / S, -np.inf)
    cosg = np.where(suspect, -np.inf, cosg)
    si, sg = np.nonzero(suspect)
    for i, g in zip(si, sg):
        sl_ = slice(g * GW, (g + 1) * GW)
        row = f1q32[i] @ f2n8[:, sl_]
        mask = np.ones(GW, bool)
        mask[np.flatnonzero((lab[sl_] == lab[i]) | (ar[sl_] == i))] = False
        cosg[i, g] = np.float64(srn1[i]) * row[mask].max() / S
    neg = np.maximum(0.0, cosg.max(axis=1))

    pos = np.clip(ps * rn1 * rn2.astype(np.float64), -1.0, 1.0)
    m = EMA * np.mean(pos - neg)
    z = S * (pos - m)
    loss = np.mean(np.log(sumoff + np.exp(z)) - z)
    return np.float32(loss)


def _unpack_st1(res):
    rn2 = np.empty(B, np.float32)
    rn1 = np.empty(B, np.float64)
    for c in range(NCORES):
        st = np.asarray(res[c]["st1"], np.float64)
        sl = slice(c * BS, (c + 1) * BS)
        rn2[sl] = st[:, 0:8].T.reshape(BS).astype(np.float32)
        rn1[sl] = st[:, 8:16].T.reshape(BS)
    return rn1, rn2


def _unpack_st2(res):
    mx = np.empty((B, NJG), np.float64)
    se = np.empty((B, NJG), np.float64)
    ps = np.empty(B, np.float64)
    for c in range(NCORES):
        st = np.asarray(res[c]["st2"], np.float64)
        # merge the split first group (jg0, m0): cols 64/65 hold its
        # panel-1 half max / sumexp
        st = st.copy()
        st[:, 0] = np.maximum(st[:, 0], st[:, 64])
        st[:, 32] = st[:, 32] + st[:, 65]
        sl = slice(c * BS, (c + 1) * BS)
        mx[sl] = st[:, 0:32].reshape(128, NJG, MT).transpose(2, 0, 1).reshape(BS, NJG)
        se[sl] = st[:, 32:64].reshape(128, NJG, MT).transpose(2, 0, 1).reshape(BS, NJG)
        ps[sl] = st[:, 66:74].T.reshape(BS)
    return mx, se, ps


def kernel(feature1, feature2, label):
    f1 = np.ascontiguousarray(np.asarray(feature1, dtype=np.float32))
    f2 = np.ascontiguousarray(np.asarray(feature2, dtype=np.float32))
    lab = np.asarray(label)

    f1b = f1.astype(ml_dtypes.bfloat16)
    f2b = f2.astype(ml_dtypes.bfloat16)

    # program 1: per-core shard norms + own-row dots
    out1 = run_bass_kernel_spmd(
        _get_prog1(), _prog1_inputs(f1b, f2b), list(range(NCORES))
    ).results
    rn1, rn2 = _unpack_st1(out1)

    # program 2: main cos/exp/max stats
    out2 = run_bass_kernel_spmd(
        _get_prog2(), _prog2_inputs(f1, f2, rn1, rn2), list(range(NCORES))
    ).results
    mx, se, ps = _unpack_st2(out2)

    return _host_combine(f1, f2, lab, rn1, rn2, ps, mx, se)
